# revision 14
# baseline (speedup 1.0000x reference)
"""Trainium2 Bass kernel for DifferentialMultiHeadSelfAttention.

Sharding: 16 heads -> 8 cores (2 heads/core, tensor parallel). Everything up
to the output Linear is head-local (GroupNorm has num_groups == n_heads, so
each head's 64 channels normalize independently). Per-head AllToAlls exchange
normalized channel slices (head-0's collective hides behind head-1's
attention), then each core computes a 256-row slice of the output Linear.
The host concatenates the slices.

Shapes (hardcoded): B=1, S=2048, E=1024, H=16, DH=64.
"""
import numpy as np

from concourse import bacc, mybir, tile
from concourse.bass_utils import run_bass_kernel_spmd

dt = mybir.dt

NCORES = 8
S = 2048
E = 1024
H = 16
DH = 64
HLOC = H // NCORES          # heads per core = 2
CLOC = HLOC * DH            # channels per core = 128
SLOC = S // NCORES          # output rows per core = 256
NT = S // 128               # 16 t-tiles
EPS = 1e-5

_PROGRAM_CACHE = {}


def _classify_mask(mask):
    """mask[s, t] bool, True = masked. Returns per-(t_tile, s_tile) state:
    0 = fully masked (skip), 1 = fully unmasked, 2 = needs mask multiply;
    plus the s-range start per t-tile and the keep (0/1) tiles in [t, s]
    layout for the state-2 blocks."""
    m = mask.reshape(NT, 128, NT, 128)  # [s_tile, s_in, t_tile, t_in]
    state = np.empty((NT, NT), dtype=np.int32)  # [t_tile, s_tile]
    keep_tiles = []
    tile_idx = -np.ones((NT, NT), dtype=np.int32)
    for t in range(NT):
        for s in range(NT):
            blk = m[s, :, t, :]  # [s_in, t_in]
            if blk.all():
                state[t, s] = 0
            elif not blk.any():
                state[t, s] = 1
            else:
                state[t, s] = 2
                tile_idx[t, s] = len(keep_tiles)
                keep_tiles.append((~blk.T).astype(np.float16))  # [t_in, s_in]
    s0 = np.full(NT, NT, dtype=np.int32)
    for t in range(NT):
        act = np.nonzero(state[t] != 0)[0]
        if len(act):
            s0[t] = act[0]
            for s in range(act[0], NT):
                if state[t, s] == 0:
                    state[t, s] = 2
                    tile_idx[t, s] = len(keep_tiles)
                    keep_tiles.append(np.zeros((128, 128), dtype=np.float16))
    if not keep_tiles:
        keep_tiles.append(np.zeros((128, 128), dtype=np.float16))
    return state, s0, tile_idx, np.stack(keep_tiles)


def _build_program(state, s0, tile_idx, n_keep, debug=False):
    nc = bacc.Bacc(None, num_devices=NCORES)

    # ---- external I/O (16-bit operands for all PE inputs) ----
    XT = nc.dram_tensor("xt", [E, S], dt.float16, kind="ExternalInput")
    WQK = nc.dram_tensor("wqk", [4, E, 128], dt.float16, kind="ExternalInput")
    WV = nc.dram_tensor("wv", [E, 128], dt.float16, kind="ExternalInput")
    BQK = nc.dram_tensor("bqk", [4, 128, 1], dt.float32, kind="ExternalInput")
    BV = nc.dram_tensor("bv", [128, 1], dt.float32, kind="ExternalInput")
    LAMN = nc.dram_tensor("lamn", [HLOC, 128, 1], dt.float32, kind="ExternalInput")
    KEEP = nc.dram_tensor("keep", [n_keep, 128, 128], dt.float16,
                          kind="ExternalInput")
    EYE = nc.dram_tensor("eye", [128, 128], dt.float32, kind="ExternalInput")
    GNW = nc.dram_tensor("gnw", [HLOC, 64, 1], dt.float32, kind="ExternalInput")
    GNB = nc.dram_tensor("gnb", [HLOC, 64, 1], dt.float32, kind="ExternalInput")
    WO = nc.dram_tensor("wo", [E, E], dt.float16, kind="ExternalInput")
    BO = nc.dram_tensor("bo", [1, E], dt.float16, kind="ExternalInput")
    ONES1 = nc.dram_tensor("ones1", [1, 128], dt.float16, kind="ExternalInput")
    OUT = nc.dram_tensor("out_slice", [SLOC, E], dt.float32, kind="ExternalOutput")
    if debug:
        DQK0 = nc.dram_tensor("d_qk0", [128, S], dt.float32, kind="ExternalOutput")
        DVT = nc.dram_tensor("d_vT", [128, S], dt.float32, kind="ExternalOutput")
        DOB = nc.dram_tensor("d_ob", [128, NT * CLOC], dt.float32,
                             kind="ExternalOutput")

    # internal DRAM: per-head AllToAll of normalized channel slices (fp16)
    a2a_in = [nc.dram_tensor(f"a2a_in{h}", [NCORES * DH, SLOC], dt.float16)
              for h in range(HLOC)]
    a2a_out = [nc.dram_tensor(f"a2a_out{h}", [NCORES * DH, SLOC], dt.float16)
               for h in range(HLOC)]
    groups = [list(range(NCORES))]

    Exp = mybir.ActivationFunctionType.Exp
    Sqrt = mybir.ActivationFunctionType.Sqrt
    Square = mybir.ActivationFunctionType.Square
    ADD = mybir.AluOpType.add
    SUB = mybir.AluOpType.subtract
    MUL = mybir.AluOpType.mult

    with tile.TileContext(nc) as tc:
        with tc.tile_pool(name="consts", bufs=1) as consts, \
             tc.tile_pool(name="qk", bufs=1) as qkp, \
             tc.tile_pool(name="vaug", bufs=1) as vaugp, \
             tc.tile_pool(name="oboth", bufs=1) as obothp:

            # ---- small constants first (tiny DMAs) ----
            bqk = consts.tile([128, 4], dt.float32, tag="bqk")
            nc.sync.dma_start(out=bqk[:], in_=BQK[:].rearrange("j p one -> p (j one)"))
            bv = consts.tile([128, 1], dt.float32, tag="bv")
            nc.sync.dma_start(out=bv[:], in_=BV[:])
            lamn = consts.tile([128, HLOC], dt.float32, tag="lamn")
            nc.sync.dma_start(out=lamn[:], in_=LAMN[:].rearrange("j p one -> p (j one)"))
            gnw2 = consts.tile([64, HLOC], dt.float32, tag="gnw2")
            nc.sync.dma_start(out=gnw2[:], in_=GNW[:].rearrange("h p one -> p (h one)"))
            gnb2 = consts.tile([64, HLOC], dt.float32, tag="gnb2")
            nc.sync.dma_start(out=gnb2[:], in_=GNB[:].rearrange("h p one -> p (h one)"))
            ones1 = consts.tile([1, 128], dt.float16, tag="ones1")
            nc.sync.dma_start(out=ones1[:], in_=ONES1[:])
            bo = consts.tile([1, E], dt.float16, tag="bo")
            nc.sync.dma_start(out=bo[:], in_=BO[:])
            ones64 = consts.tile([1, 64], dt.float32, tag="ones64")
            nc.vector.memset(ones64[:], 1.0)
            ones_col = consts.tile([128, 1], dt.float32, tag="ones_col")
            nc.vector.memset(ones_col[:], 1.0)
            epsc = consts.tile([1, 1], dt.float32, tag="epsc")
            nc.vector.memset(epsc[:], EPS)
            eye = consts.tile([128, 128], dt.float32, tag="eye")
            keep16 = consts.tile([128, n_keep, 128], dt.float16, tag="keep16")
            wo = consts.tile([128, 8, E], dt.float16, tag="wo")

            # persistent activation tiles
            qk_sb = [qkp.tile([128, S], dt.float16, tag=f"qk{j}", name=f"qk{j}")
                     for j in range(4)]
            v_aug = [vaugp.tile([128, NT, 65], dt.float16, tag=f"va{h}", name=f"va{h}")
                     for h in range(HLOC)]
            o_both = obothp.tile([128, NT * CLOC], dt.float32, tag="o_both")
            tmp_o = obothp.tile([128, NT * DH], dt.float32, tag="tmp_o")
            sq_scr = obothp.tile([128, NT * DH], dt.float32, tag="sq_scr")
            stat4 = obothp.tile([128, 4], dt.float32, tag="stat4")
            ssum = obothp.tile([128, HLOC, NT], dt.float32, tag="ssum")
            ssq = obothp.tile([128, HLOC, NT], dt.float32, tag="ssq")
            xn_h = [obothp.tile([64, S], dt.float16, tag=f"xnh{h}", name=f"xnh{h}")
                    for h in range(HLOC)]

            # ================= phase 1: projections =================
            with tc.tile_pool(name="proj", bufs=1) as projp:
                wqk = projp.tile([128, 4, 8, 128], dt.float16, tag="wqk")
                nc.sync.dma_start(
                    out=wqk[:, 0:1, :, :],
                    in_=WQK[0:1].rearrange("j (e p) c -> p j e c", p=128))
                xt = projp.tile([128, 8, S], dt.float16, tag="xt")
                for e in range(8):
                    nc.sync.dma_start(out=xt[:, e, :],
                                      in_=XT[128 * e:128 * (e + 1), :])
                nc.sync.dma_start(
                    out=wqk[:, 1:4, :, :],
                    in_=WQK[1:4].rearrange("j (e p) c -> p j e c", p=128))
                wv = projp.tile([128, 8, 128], dt.float16, tag="wv")
                nc.sync.dma_start(out=wv[:],
                                  in_=WV[:].rearrange("(e p) c -> p e c", p=128))
                # bulky constants after the projection operands
                nc.sync.dma_start(out=keep16[:],
                                  in_=KEEP[:].rearrange("n p f -> p n f"))
                nc.sync.dma_start(out=eye[:], in_=EYE[:])
                vT = projp.tile([128, S], dt.float32, tag="vT")

                with tc.tile_pool(name="proj_ps", bufs=4, space="PSUM") as proj_ps:
                    for j in range(4):  # qpair-h0, kpair-h0, qpair-h1, kpair-h1
                        for sc in range(4):
                            ps = proj_ps.tile([128, 512], dt.float32, tag="pps")
                            for e in range(8):
                                nc.tensor.matmul(ps[:], wqk[:, j, e, :],
                                                 xt[:, e, 512 * sc:512 * (sc + 1)],
                                                 start=(e == 0), stop=(e == 7))
                            nc.vector.tensor_scalar(
                                qk_sb[j][:, 512 * sc:512 * (sc + 1)],
                                ps[:], bqk[:, j:j + 1], None, ADD)
                    for sc in range(4):
                        ps = proj_ps.tile([128, 512], dt.float32, tag="pps")
                        for e in range(8):
                            nc.tensor.matmul(ps[:], wv[:, e, :],
                                             xt[:, e, 512 * sc:512 * (sc + 1)],
                                             start=(e == 0), stop=(e == 7))
                        nc.vector.tensor_scalar(vT[:, 512 * sc:512 * (sc + 1)],
                                                ps[:], bv[:], None, ADD)

                    # ---- phase 2: transpose vT -> v_aug (fp16 + ones col) ----
                    with tc.tile_pool(name="vt_ps", bufs=4, space="PSUM") as vt_ps:
                        for h in range(HLOC):
                            for t in range(NT):
                                nc.vector.memset(v_aug[h][:, t, 64:65], 1.0)
                                ps = vt_ps.tile([128, 64], dt.float32, tag="vtps")
                                nc.tensor.transpose(
                                    ps[:],
                                    vT[64 * h:64 * (h + 1), 128 * t:128 * (t + 1)],
                                    eye[64 * h:64 * (h + 1), 64 * h:64 * (h + 1)])
                                nc.any.tensor_copy(v_aug[h][:, t, 0:64], ps[:])

            # load wo during the attention phase (off the critical DMA path)
            nc.sync.dma_start(out=wo[:], in_=WO[:].rearrange("(j p) e -> p j e", p=128))

            if debug:
                dbg_qk0 = obothp.tile([128, S], dt.float32, tag="dbg_qk0")
                nc.vector.tensor_copy(dbg_qk0[:], qk_sb[0][:])
                nc.sync.dma_start(out=DQK0[:], in_=dbg_qk0[:])
                dbg_va = obothp.tile([128, S], dt.float32, tag="dbg_va")
                for t in range(NT):
                    nc.vector.tensor_copy(dbg_va[:, 128 * t:128 * t + 64],
                                          v_aug[0][:, t, 0:64])
                nc.sync.dma_start(out=DVT[:], in_=dbg_va[:])

            # ================= phase 3: attention =================
            with tc.tile_pool(name="exp", bufs=1) as expp, \
                 tc.tile_pool(name="rr", bufs=8) as rrp, \
                 tc.tile_pool(name="sc_ps", bufs=3, space="PSUM") as sc_ps, \
                 tc.tile_pool(name="o_ps", bufs=2, space="PSUM") as o_ps:
                for h in range(HLOC):
                    qT = qk_sb[2 * h]      # [0:64]=sub-1 dims, [64:128]=sub-2
                    kT = qk_sb[2 * h + 1]
                    # scores + exp, both matrices interleaved per t so that
                    # consecutive matmuls alternate PE row-groups (LDW overlap)
                    exp_tm = [[None] * NT, [None] * NT]
                    for t in range(NT):
                        if s0[t] >= NT:
                            continue
                        base = 128 * int(s0[t])
                        for m in range(2):
                            et = expp.tile([128, S - base], dt.float16,
                                           tag=f"exp{t}m{m}", name=f"exp{t}m{m}",
                                           bufs=2)
                            exp_tm[m][t] = et
                        pos = base
                        while pos < S:
                            ln = min(1024, S - pos)
                            pss = [sc_ps.tile([128, 1024], dt.float32, tag="scps",
                                              name="scps")
                                   for _ in range(2)]
                            for off in range(0, ln, 512):
                                w = min(512, ln - off)
                                for m in range(2):
                                    p0, p1 = 64 * m, 64 * (m + 1)
                                    for th in range(2):
                                        nc.tensor.matmul(
                                            pss[m][64 * th:64 * (th + 1),
                                                   off:off + w],
                                            kT[p0:p1, 128 * t + 64 * th:
                                               128 * t + 64 * (th + 1)],
                                            qT[p0:p1, pos + off:pos + off + w],
                                            start=True, stop=True)
                            for m in range(2):
                                nc.scalar.activation(
                                    exp_tm[m][t][:, pos - base:pos - base + ln],
                                    pss[m][:, 0:ln], Exp, scale=0.125)
                            pos += ln
                        for s in range(int(s0[t]), NT):
                            ki = int(tile_idx[t, s])
                            if ki >= 0:
                                for m in range(2):
                                    blk = exp_tm[m][t][:, 128 * s - base:
                                                       128 * (s + 1) - base]
                                    nc.vector.tensor_tensor(
                                        blk, blk, keep16[:, ki, :], MUL)
                        for sv in range(NT):
                            ts = [tt for tt in range(NT)
                                  if s0[tt] <= sv and state[tt, sv] != 0]
                            if not ts or max(ts) != t:
                                continue
                            for m in range(2):
                                ops = o_ps.tile([128, 65], dt.float32, tag="ops")
                                for i, tt in enumerate(ts):
                                    b = 128 * int(s0[tt])
                                    nc.tensor.matmul(
                                        ops[:],
                                        exp_tm[m][tt][:, 128 * sv - b:
                                                      128 * (sv + 1) - b],
                                        v_aug[h][:, tt, :],
                                        start=(i == 0), stop=(i == len(ts) - 1))
                                rec = rrp.tile([128, 1], dt.float32, tag="rec")
                                nc.vector.reciprocal(rec[:], ops[:, 64:65])
                                if m == 0:
                                    nc.vector.tensor_scalar(
                                        tmp_o[:, DH * sv:DH * (sv + 1)],
                                        ops[:, 0:64], rec[:], None, MUL)
                                else:
                                    rl = rrp.tile([128, 1], dt.float32, tag="rl")
                                    nc.vector.tensor_tensor(rl[:], rec[:],
                                                            lamn[:, h:h + 1], MUL)
                                    ob = o_both[:, CLOC * sv + DH * h:
                                                CLOC * sv + DH * (h + 1)]
                                    nc.vector.scalar_tensor_tensor(
                                        ob, ops[:, 0:64], rl[:],
                                        tmp_o[:, DH * sv:DH * (sv + 1)], MUL, ADD,
                                        accum_out=ssum[:, h, sv:sv + 1])
                                    nc.vector.scalar_tensor_tensor(
                                        sq_scr[:, DH * sv:DH * (sv + 1)],
                                        ob, 1.0, ob, mybir.AluOpType.bypass, MUL,
                                        accum_out=ssq[:, h, sv:sv + 1])
                    # head finished: stats -> GN affine -> fused apply into the
                    # transpose evacuation -> per-head AllToAll
                    nc.vector.tensor_reduce(stat4[:, 2 * h:2 * h + 1],
                                            ssum[:, h, :], mybir.AxisListType.X,
                                            ADD)
                    nc.vector.tensor_reduce(stat4[:, 2 * h + 1:2 * h + 2],
                                            ssq[:, h, :], mybir.AxisListType.X,
                                            ADD)
                    red = o_ps.tile([1, 2], dt.float32, tag="ops", name="red")
                    nc.tensor.matmul(red[:], ones_col[:], stat4[:, 2 * h:2 * h + 2],
                                     start=True, stop=True)
                    scal = rrp.tile([1, 7], dt.float32, tag="scal")
                    n_inv = 1.0 / (S * DH)
                    # cols: mean, E[x^2], mean^2, var, std, inv, negmu
                    nc.vector.tensor_scalar(scal[:, 0:2], red[:], n_inv, None, MUL)
                    nc.vector.tensor_tensor(scal[:, 2:3], scal[:, 0:1],
                                            scal[:, 0:1], MUL)
                    nc.vector.tensor_tensor(scal[:, 3:4], scal[:, 1:2],
                                            scal[:, 2:3], SUB)
                    nc.scalar.activation(scal[:, 4:5], scal[:, 3:4],
                                         mybir.ActivationFunctionType.Ln,
                                         bias=epsc[0:1, 0:1])
                    nc.scalar.activation(scal[:, 5:6], scal[:, 4:5], Exp,
                                         scale=-0.5)
                    nc.vector.tensor_scalar(scal[:, 6:7], scal[:, 0:1], -1.0,
                                            None, MUL)
                    bc = o_ps.tile([64, 2], dt.float32, tag="ops", name="bc")
                    nc.tensor.matmul(bc[:], ones64[:], scal[:, 5:7],
                                     start=True, stop=True)
                    a_h = rrp.tile([64, 1], dt.float32, tag="a_h")
                    nc.vector.tensor_tensor(a_h[:], gnw2[:, h:h + 1], bc[:, 0:1],
                                            MUL)
                    b_h = rrp.tile([64, 1], dt.float32, tag="b_h")
                    nc.vector.scalar_tensor_tensor(b_h[:], a_h[:], bc[:, 1:2],
                                                   gnb2[:, h:h + 1], MUL, ADD)
                    for t in range(NT):
                        ps = sc_ps.tile([64, 128], dt.float32, tag="scps", name="xpps")
                        nc.tensor.transpose(
                            ps[:],
                            o_both[:, CLOC * t + DH * h:CLOC * t + DH * (h + 1)],
                            eye[:])
                        nc.vector.tensor_scalar(xn_h[h][:, 128 * t:128 * (t + 1)],
                                                ps[:], a_h[:], b_h[:], MUL, ADD)
                    for i in range(NCORES):
                        nc.sync.dma_start(out=a2a_in[h][64 * i:64 * (i + 1), :],
                                          in_=xn_h[h][:, SLOC * i:SLOC * (i + 1)])
                    nc.gpsimd.collective_compute(
                        "AllToAll", mybir.AluOpType.bypass, replica_groups=groups,
                        ins=[a2a_in[h][:]], outs=[a2a_out[h][:]])

            if debug:
                nc.sync.dma_start(out=DOB[:], in_=o_both[:])

            # ============ phase 4: output Linear on exchanged slices ==========
            with tc.tile_pool(name="fin", bufs=1) as finp:
                xa = finp.tile([128, NCORES, SLOC], dt.float16, tag="xa")
                for h in range(HLOC):
                    nc.sync.dma_start(
                        out=xa[64 * h:64 * (h + 1), :, :],
                        in_=a2a_out[h][:].rearrange("(j p) s -> p j s", p=64))
                out_sb = finp.tile([128, 2, E], dt.float32, tag="out_sb")
                with tc.tile_pool(name="f_ps", bufs=4, space="PSUM") as f_ps:
                    for sh in range(2):   # two 128-row halves of the 256-row slice
                        for ec in range(2):  # two 512-col chunks of E
                            ps = f_ps.tile([128, 512], dt.float32, tag="f_ps")
                            for j in range(NCORES):
                                nc.tensor.matmul(
                                    ps[:], xa[:, j, 128 * sh:128 * (sh + 1)],
                                    wo[:, j, 512 * ec:512 * (ec + 1)],
                                    start=(j == 0), stop=False)
                            nc.tensor.matmul(ps[:], ones1[:],
                                             bo[:, 512 * ec:512 * (ec + 1)],
                                             start=False, stop=True)
                            nc.any.tensor_copy(out_sb[:, sh, 512 * ec:512 * (ec + 1)],
                                               ps[:])
                nc.sync.dma_start(out=OUT[:].rearrange("(sh p) e -> p sh e", sh=2),
                                  in_=out_sb[:])
    nc.finalize()
    return nc


def _prep_inputs(x, mask, Wq1, bq1, Wq2, bq2, Wk1, bk1, Wk2, bk2, Wv, bv,
                 lam, gn_w, gn_b, Wo, bo):
    import os
    f32 = np.float32
    f16 = np.float16
    x = np.asarray(x, f32).reshape(S, E)
    mask = np.asarray(mask, bool)
    state, s0, tile_idx, keep = _classify_mask(mask)
    dbg = bool(os.environ.get("KERNEL_DEBUG"))
    key = (state.tobytes(), s0.tobytes(), dbg)
    if key not in _PROGRAM_CACHE:
        _PROGRAM_CACHE[key] = _build_program(state, s0, tile_idx, len(keep),
                                             debug=dbg)
    nc = _PROGRAM_CACHE[key]

    xT = np.ascontiguousarray(x.T).astype(f16)
    woT = np.ascontiguousarray(np.asarray(Wo, f32).T).astype(f16)
    eye = np.eye(128, dtype=f32)
    ones1 = np.ones((1, 128), dtype=f16)
    bo_r = np.asarray(bo, f32).reshape(1, E).astype(f16)

    Wq1, Wq2, Wk1, Wk2, Wv = (np.asarray(a, f32) for a in (Wq1, Wq2, Wk1, Wk2, Wv))
    bq1, bq2, bk1, bk2, bv = (np.asarray(a, f32) for a in (bq1, bq2, bk1, bk2, bv))
    lam = np.asarray(lam, f32)
    gn_w = np.asarray(gn_w, f32)
    gn_b = np.asarray(gn_b, f32)

    in_maps = []
    for c in range(NCORES):
        h0, h1 = 2 * c, 2 * c + 1
        wqk = np.stack([
            np.concatenate([Wq1[h0].T, Wq2[h0].T], axis=1),
            np.concatenate([Wk1[h0].T, Wk2[h0].T], axis=1),
            np.concatenate([Wq1[h1].T, Wq2[h1].T], axis=1),
            np.concatenate([Wk1[h1].T, Wk2[h1].T], axis=1)]).astype(f16)
        wv = np.concatenate([Wv[h0].T, Wv[h1].T], axis=1).astype(f16)
        bqk = np.stack([
            np.concatenate([bq1[h0], bq2[h0]]),
            np.concatenate([bk1[h0], bk2[h0]]),
            np.concatenate([bq1[h1], bq2[h1]]),
            np.concatenate([bk1[h1], bk2[h1]])])[..., None]
        bvv = np.concatenate([bv[h0], bv[h1]])[:, None]
        lamn = np.stack([np.full((128, 1), -lam[h0], f32),
                         np.full((128, 1), -lam[h1], f32)])
        gnw = np.stack([gn_w[DH * (2 * c + h):DH * (2 * c + h + 1), None]
                        for h in range(HLOC)])
        gnb = np.stack([gn_b[DH * (2 * c + h):DH * (2 * c + h + 1), None]
                        for h in range(HLOC)])
        in_maps.append({
            "xt": xT, "wqk": np.ascontiguousarray(wqk),
            "wv": np.ascontiguousarray(wv),
            "bqk": np.ascontiguousarray(bqk), "bv": np.ascontiguousarray(bvv),
            "lamn": lamn, "keep": keep, "eye": eye,
            "gnw": np.ascontiguousarray(gnw), "gnb": np.ascontiguousarray(gnb),
            "wo": woT, "bo": bo_r, "ones1": ones1,
        })
    return nc, in_maps


def kernel(**inputs):
    nc, in_maps = _prep_inputs(**inputs)
    res = run_bass_kernel_spmd(nc, in_maps, list(range(NCORES)))
    out = np.concatenate([res.results[c]["out_slice"] for c in range(NCORES)],
                         axis=0)
    return out.reshape(1, S, E).astype(np.float32)
